# revision 25
# baseline (speedup 1.0000x reference)
"""Local (sliding-window) self-attention Bass kernel for 8 TRN2 NeuronCores.

Problem: B=4, T=4096, C=512, H=8 heads, head_dim=64, window=15.
Sharding: 8 cores = batch(4) x seq-halves(2). Each core processes 2048 query
tokens of one batch element; its x chunk carries a 7-token halo on each side
(zero-padded at sequence edges, matching the reference's jnp.pad semantics),
padded to 2080 rows.

Per-core dataflow (bf16 matmuls, fp32 PSUM accumulation):
  x bf16 [NKV,C] --PE matmul vs diag(mask)--> xT (feature-major, mask fused)
  qT/kT feature-major: W-stationary matmuls + bias on ACT
  v per 114-token block: x-stationary matmul, bias on DVE, stored head-major
    with a ones column per head (65-stride) for the softmax denominator
  attention per 114-token block (kv extent 128, no K-split):
    scores kv-major [128kv, W]: 1 matmul/head -> exp on ACT -> band mask DVE
    AV token-major [W, 65]/head: lhsT=alphaT, rhs=v|1 -> denom in col 64
    reciprocal + per-partition normalize on DVE -> attn_tok bf16
    PE is_transpose -> attnT feature-major
  proj: attnT-stationary matmuls; bias add + mask on DVE; bf16 out
"""

import math
from contextlib import ExitStack

import ml_dtypes
import numpy as np

import concourse.bacc as bacc
import concourse.bass as bass
import concourse.mybir as mybir
import concourse.tile as tile
from concourse import bass_utils

B, T, C, H, WIN = 4, 4096, 512, 8, 15
D = C // H            # 64
PAD = WIN // 2        # 7
NTOK = T // 2         # 2048 query tokens per core
NKV = 2080            # kv rows per core: 7 + 2048 + 7 = 2062, padded to 2080
QB = 114              # query block (kv extent = QB + WIN - 1 = 128)
NQB = 18              # 17 full blocks + one 110-wide block
KCH = [512, 512, 512, 512, 32]  # kv chunks for the k GEMM
SCALE = math.log(WIN) / D
F32 = mybir.dt.float32
BF16 = mybir.dt.bfloat16


def _band4() -> np.ndarray:
    """[128, 4*114] bf16: band[p, h*114+j] = 1 iff 0 <= p - j <= 14."""
    p = np.arange(128)[:, None]
    j = np.arange(QB)[None, :]
    band = ((p >= j) & (p <= j + WIN - 1)).astype(ml_dtypes.bfloat16)
    return np.ascontiguousarray(np.tile(band, (1, 4)))


def _blk_w(b: int) -> int:
    return min(QB, NTOK - b * QB)


def build_program() -> bacc.Bacc:
    nc = bacc.Bacc("TRN2", target_bir_lowering=False, debug=False,
                   enable_asserts=False, num_devices=8)

    xd = nc.dram_tensor("x", [NKV, C], BF16, kind="ExternalInput").ap()
    maskd = nc.dram_tensor("mask", [17 * 128], F32, kind="ExternalInput").ap()
    wqd = nc.dram_tensor("wq", [C, C], BF16, kind="ExternalInput").ap()
    wkd = nc.dram_tensor("wk", [C, C], BF16, kind="ExternalInput").ap()
    wvd = nc.dram_tensor("wv", [C, C], BF16, kind="ExternalInput").ap()
    wpd = nc.dram_tensor("wp", [C, C], BF16, kind="ExternalInput").ap()
    bqd = nc.dram_tensor("bq", [C], F32, kind="ExternalInput").ap()
    bkd = nc.dram_tensor("bk", [C], F32, kind="ExternalInput").ap()
    bvd = nc.dram_tensor("bv", [C], F32, kind="ExternalInput").ap()
    bpd = nc.dram_tensor("bp", [C], F32, kind="ExternalInput").ap()
    bandd = nc.dram_tensor("band4", [128, 4 * QB], BF16, kind="ExternalInput").ap()
    eyed = nc.dram_tensor("eye", [128, 128], BF16, kind="ExternalInput").ap()
    outd = nc.dram_tensor("out", [NTOK, C], BF16, kind="ExternalOutput").ap()

    with tile.TileContext(nc) as tc, ExitStack() as ctx:
        sb = ctx.enter_context(tc.tile_pool(name="sb", bufs=1))
        sb_x = ctx.enter_context(tc.tile_pool(name="sb_x", bufs=5))
        sb_e = ctx.enter_context(tc.tile_pool(name="sb_e", bufs=4))
        sb_al = ctx.enter_context(tc.tile_pool(name="sb_al", bufs=4))
        sb_at = ctx.enter_context(tc.tile_pool(name="sb_at", bufs=3))
        sb_rd = ctx.enter_context(tc.tile_pool(name="sb_rd", bufs=4))
        sb_o = ctx.enter_context(tc.tile_pool(name="sb_o", bufs=3))
        pp_big = ctx.enter_context(tc.tile_pool(name="pp_big", bufs=3, space="PSUM"))
        pp_sc = ctx.enter_context(tc.tile_pool(name="pp_sc", bufs=2, space="PSUM"))
        pp_at = ctx.enter_context(tc.tile_pool(name="pp_at", bufs=2, space="PSUM"))
        pp_tr = ctx.enter_context(tc.tile_pool(name="pp_tr", bufs=1, space="PSUM"))

        # ---- persistent SBUF ----
        xT = sb.tile([128, 4 * NKV], BF16, tag="xT")     # feature-major masked x
        qT = sb.tile([128, 8 * NTOK], BF16, tag="qT")  # per-head, zero-padded halves
        kT = sb.tile([128, 4 * NKV], BF16, tag="kT")
        aT = sb.tile([128, 4 * NTOK], BF16, tag="aT")    # feature-major attn
        vb = [sb.tile([128, 8 * 65], BF16, tag=f"vb{b}", name=f"vb{b}")
              for b in range(NQB)]
        wq_t = sb.tile([128, 4 * C], BF16, tag="wq")
        wk_t = sb.tile([128, 4 * C], BF16, tag="wk")
        wv_t = sb.tile([128, 4 * C], BF16, tag="wv")
        wp_t = sb.tile([128, 4 * C], BF16, tag="wp")
        bq_t = sb.tile([128, 4], F32, tag="bq")
        bk_t = sb.tile([128, 4], F32, tag="bk")
        bpB = sb.tile([128, C], F32, tag="bpB")
        mcol = sb.tile([128, 17], F32, tag="mcol")       # mask per x tile
        mq = sb.tile([128, 16], F32, tag="mq")           # query mask per 128-blk
        band = sb.tile([128, 4 * QB], BF16, tag="band")
        eye = sb.tile([128, 128], BF16, tag="eye")

        # ---- prologue DMAs (sync queue: consts + weights; scalar queue: x) ----
        nc.sync.dma_start(band[:], bandd)
        nc.sync.dma_start(eye[:], eyed)
        nc.sync.dma_start(mcol[:], maskd.rearrange("(a b) -> b a", b=128))
        nc.sync.dma_start(mq[:], maskd[PAD:PAD + NTOK].rearrange("(a b) -> b a", b=128))
        nc.sync.dma_start(bq_t[:], bqd.rearrange("(a b) -> b a", b=128))
        nc.sync.dma_start(bk_t[:], bkd.rearrange("(a b) -> b a", b=128))
        nc.sync.dma_start(bpB[:], bpd[None, :].broadcast_to((128, C)))
        xf_tiles = []
        for g in range(5):
            xf = sb_x.tile([128, 4 * C], BF16, tag="xf", name=f"xf{g}")
            if g < 4:
                nc.sync.dma_start(
                    xf[:],
                    xd[g * 512:(g + 1) * 512, :].rearrange("(a p) c -> p a c", p=128))
            else:
                nc.sync.dma_start(xf[0:32, 0:C], xd[2048:NKV, :])
            xf_tiles.append(xf)
        for ci in range(4):
            nc.sync.dma_start(wq_t[:, ci * C:(ci + 1) * C], wqd[ci * 128:(ci + 1) * 128, :])
            nc.sync.dma_start(wk_t[:, ci * C:(ci + 1) * C], wkd[ci * 128:(ci + 1) * 128, :])
            nc.sync.dma_start(wv_t[:, ci * C:(ci + 1) * C], wvd[ci * 128:(ci + 1) * 128, :])
            nc.sync.dma_start(wp_t[:, ci * C:(ci + 1) * C], wpd[ci * 128:(ci + 1) * 128, :])
        # ones columns (softmax denominator) in every v tile
        for b in range(NQB):
            nc.gpsimd.memset(vb[b].rearrange("p (h w) -> p h w", w=65)[:, :, 64:65], 1.0)
        # zero the unused partition half of each head's qT section
        for h in range(8):
            p0 = 0 if h % 2 else 64
            nc.gpsimd.memset(qT[p0:p0 + 64, h * NTOK:(h + 1) * NTOK], 0.0)

        # ---- x in + masked transpose (PE matmul vs diag(mask)) ----
        def emit_xtile(t: int):
            r0 = t * 128
            rows = min(128, NKV - r0)
            xf = xf_tiles[t // 4]
            c0 = (t % 4) * C
            dg = sb_x.tile([128, 128], BF16, tag="dg")
            nc.vector.tensor_scalar_mul(dg[:rows, :rows], eye[:rows, :rows],
                                        mcol[:rows, t:t + 1])
            ps = pp_big.tile([128, 512], F32, tag="big")
            for ci in range(4):
                nc.tensor.matmul(ps[:, ci * 128:ci * 128 + rows],
                                 xf[:rows, c0 + ci * 128:c0 + (ci + 1) * 128],
                                 dg[:rows, :rows], start=True, stop=True)
            nc.scalar.activation(
                xT.rearrange("p (c n) -> p c n", c=4)[:, :, r0:r0 + rows],
                ps.rearrange("p (c n) -> p c n", c=4)[:, :, 0:rows],
                mybir.ActivationFunctionType.Copy)

        def emit_q(ch: int):
            t0 = ch * 512
            for co in range(4):
                ps = pp_big.tile([128, 512], F32, tag="big")
                for ci in range(4):
                    nc.tensor.matmul(
                        ps[:], wq_t[:, ci * C + co * 128:ci * C + (co + 1) * 128],
                        xT[:, ci * NKV + PAD + t0:ci * NKV + PAD + t0 + 512],
                        start=(ci == 0), stop=(ci == 3))
                for e in range(2):
                    h = 2 * co + e
                    p0 = 64 * e
                    nc.scalar.activation(
                        qT[p0:p0 + 64, h * NTOK + t0:h * NTOK + t0 + 512],
                        ps[p0:p0 + 64, :],
                        mybir.ActivationFunctionType.Identity,
                        bias=bq_t[p0:p0 + 64, co:co + 1])

        def emit_k(ch: int):
            t0 = sum(KCH[:ch])
            w = KCH[ch]
            for co in range(4):
                ps = pp_big.tile([128, 512], F32, tag="big")
                for ci in range(4):
                    nc.tensor.matmul(
                        ps[:, 0:w], wk_t[:, ci * C + co * 128:ci * C + (co + 1) * 128],
                        xT[:, ci * NKV + t0:ci * NKV + t0 + w],
                        start=(ci == 0), stop=(ci == 3))
                nc.scalar.activation(kT[:, co * NKV + t0:co * NKV + t0 + w], ps[:, 0:w],
                                     mybir.ActivationFunctionType.Identity,
                                     bias=bk_t[:, co:co + 1])

        def emit_v(b: int):
            kv0 = b * QB
            ps = pp_big.tile([128, 512], F32, tag="big")
            for ci in range(4):
                nc.tensor.matmul(ps[:], xT[:, ci * NKV + kv0:ci * NKV + kv0 + 128],
                                 wv_t[:, ci * C:(ci + 1) * C],
                                 start=(ci == 0), stop=(ci == 3))
            nc.vector.tensor_copy(
                vb[b].rearrange("p (h w) -> p h w", w=65)[:, :, 0:64],
                ps.rearrange("p (h w) -> p h w", w=64))

        sc_tiles: dict = {}
        al_tiles: dict = {}
        at_tiles: dict = {}
        rd_tiles: dict = {}
        atok_tiles: dict = {}

        def emit_sc(b: int):
            import os
            KSC = int(os.environ.get("KSC", "3"))
            kv0, q0, w = b * QB, b * QB, _blk_w(b)
            sc_tiles[b] = []
            al_tiles[b] = []
            for g in range(2):
                ps = pp_sc.tile([128, 512], F32, tag="sc")
                for pp in range(2):          # head pair co = g*2 + pp
                    co = g * 2 + pp
                    nc.tensor.matmul(
                        ps[:, pp * 256:pp * 256 + 2 * w],
                        kT[:, co * NKV + kv0:co * NKV + kv0 + 128],
                        qT.rearrange("p (h n) -> p h n", h=8)[:, 2 * co:2 * co + 2, q0:q0 + w],
                        start=True, stop=True)
                ex = sb_e.tile([128, 4 * QB], BF16, tag="ex")
                if KSC >= 2:
                    nc.scalar.activation(
                        ex[:, 0:4 * w].rearrange("p (g n) -> p g n", n=2 * w),
                        ps.rearrange("p (g n) -> p g n", g=2)[:, :, 0:2 * w],
                        mybir.ActivationFunctionType.Exp, scale=SCALE)
                al = sb_al.tile([128, 4 * QB], BF16, tag="al")
                if KSC >= 3:
                    nc.vector.tensor_mul(
                        al.rearrange("p (h w) -> p h w", h=4)[:, :, 0:w],
                        ex[:, 0:4 * w].rearrange("p (h w) -> p h w", w=w),
                        band.rearrange("p (h w) -> p h w", h=4)[:, :, 0:w])
                sc_tiles[b].append(ps)
                al_tiles[b].append(al)

        def emit_av(b: int):
            w = _blk_w(b)
            at_tiles[b] = []
            rd = sb_rd.tile([128, 8], F32, tag="rd")
            atok = sb_at.tile([128, 512], BF16, tag="atok")
            for g in range(2):
                al = al_tiles[b][g]
                ps = pp_at.tile([128, 512], F32, tag="at")
                for hh in range(4):
                    h = g * 4 + hh
                    nc.tensor.matmul(
                        ps[0:w, hh * 128:hh * 128 + 65],
                        al[:, hh * QB:hh * QB + w],
                        vb[b][:, h * 65:(h + 1) * 65],
                        start=True, stop=True)
                nc.vector.reciprocal(
                    rd[0:w, g * 4:(g + 1) * 4],
                    ps.rearrange("p (h w) -> p h w", w=128)[0:w, :, 64])
                at_tiles[b].append(ps)
            for g in range(2):
                nc.vector.tensor_tensor(
                    atok[0:w, g * 256:(g + 1) * 256].rearrange("p (h w) -> p h w", w=64),
                    at_tiles[b][g].rearrange("p (h w) -> p h w", w=128)[0:w, :, 0:64],
                    rd.rearrange("p (h one) -> p h one", one=1)[0:w, 4 * g:4 * g + 4]
                      .broadcast_to((w, 4, 64)),
                    op=mybir.AluOpType.mult)
            rd_tiles[b] = rd
            atok_tiles[b] = atok
            del al_tiles[b]

        def emit_tr(b: int):
            w = _blk_w(b)
            q0 = b * QB
            atok = atok_tiles.pop(b)
            ps = pp_tr.tile([128, 1024], BF16, tag="tr")
            for ci in range(4):
                nc.tensor.transpose(ps[:, ci * 256:ci * 256 + w],
                                    atok[0:w, ci * 128:(ci + 1) * 128],
                                    eye[0:w, 0:w])
            nc.scalar.activation(
                aT.rearrange("p (c n) -> p c n", c=4)[:, :, q0:q0 + w],
                ps.rearrange("p (c n) -> p c n", c=4)[:, :, 0:w],
                mybir.ActivationFunctionType.Copy)

        ot_cur: list = [None]

        def emit_proj(j: int):
            t0 = j * 128
            ps = pp_big.tile([128, 512], F32, tag="big")
            nc.vector.tensor_copy(ps[:], bpB[:])
            for ci in range(4):
                nc.tensor.matmul(ps[:], aT[:, ci * NTOK + t0:ci * NTOK + t0 + 128],
                                 wp_t[:, ci * C:(ci + 1) * C],
                                 start=False, stop=(ci == 3), skip_group_check=True)
            if j % 4 == 0:
                ot_cur[0] = sb_o.tile([128, 4 * C], BF16, tag="ot", name=f"ot{j // 4}")
            ot = ot_cur[0]
            nc.vector.tensor_scalar_mul(ot[:, (j % 4) * C:(j % 4 + 1) * C],
                                        ps[:], mq[:, j:j + 1])
            if j % 4 == 3:
                j0 = j - 3
                nc.sync.dma_start(
                    outd[j0 * 128:(j0 + 4) * 128, :]
                        .rearrange("(a p) c -> p a c", p=128),
                    ot.rearrange("p (a c) -> p a c", a=4))

        # ---- schedule ----
        import os
        PH = int(os.environ.get("KPH", "7"))
        for t in range(17):
            emit_xtile(t)
            if PH >= 2 and t in (4, 8, 12, 16):
                emit_q(t // 4 - 1)
        kv_done = 0
        k_next = 0
        proj_next = 0
        for b in range(NQB + 2):
            if b >= 2 and PH >= 6:
                emit_tr(b - 2)
            if b < NQB and PH >= 2:
                need = min(b * QB + 128, NKV)
                while kv_done < need:
                    emit_k(k_next)
                    kv_done += KCH[k_next]
                    k_next += 1
                if PH >= 4:
                    emit_sc(b)
            if 1 <= b <= NQB and PH >= 5:
                emit_av(b - 1)
            if b == 0 and PH >= 3:
                emit_v(0)
            if b + 1 < NQB and PH >= 3:
                emit_v(b + 1)
            if b >= 2 and PH >= 6:
                done = min((b - 1) * QB, NTOK)
                while proj_next < 16 and (proj_next + 1) * 128 <= done and PH >= 7:
                    emit_proj(proj_next)
                    proj_next += 1
        while proj_next < 16 and PH >= 7:
            emit_proj(proj_next)
            proj_next += 1
        if PH < 7:
            dbg = sb.tile([128, C], BF16, tag="dbg")
            nc.vector.tensor_copy(dbg[:], bpB[:])
            nc.sync.dma_start(outd[0:128, :], dbg[:])

    nc.compile()
    return nc


_CACHE: dict = {}


def _get_program() -> bacc.Bacc:
    if "nc" not in _CACHE:
        _CACHE["nc"] = build_program()
    return _CACHE["nc"]


def _core_inputs(x, mask, Wq, bq, Wkv, bkv, Wproj, bproj, core: int) -> dict:
    b, h = divmod(core, 2)
    s = h * NTOK
    xc = np.zeros((NKV, C), np.float32)
    mc = np.zeros((17 * 128,), np.float32)
    lo, hi = max(0, s - PAD), min(T, s + NTOK + PAD)
    xc[lo - (s - PAD):lo - (s - PAD) + hi - lo] = x[b, lo:hi]
    mc[lo - (s - PAD):lo - (s - PAD) + hi - lo] = mask[b, lo:hi]
    bf = ml_dtypes.bfloat16
    return {
        "x": xc.astype(bf), "mask": mc,
        "wq": np.asarray(Wq, np.float32).astype(bf),
        "wk": np.ascontiguousarray(np.asarray(Wkv, np.float32)[:, :C]).astype(bf),
        "wv": np.ascontiguousarray(np.asarray(Wkv, np.float32)[:, C:]).astype(bf),
        "wp": np.asarray(Wproj, np.float32).astype(bf),
        "bq": np.asarray(bq, np.float32),
        "bk": np.ascontiguousarray(np.asarray(bkv, np.float32)[:C]),
        "bv": np.ascontiguousarray(np.asarray(bkv, np.float32)[C:]),
        "bp": (np.asarray(bkv, np.float32)[C:] @ np.asarray(Wproj, np.float32)
               + np.asarray(bproj, np.float32)),
        "band4": _band4(),
        "eye": np.ascontiguousarray(np.eye(128, dtype=bf)),
    }


def kernel(x, mask, Wq, bq, Wkv, bkv, Wproj, bproj) -> np.ndarray:
    x = np.asarray(x, np.float32)
    mask = np.asarray(mask, np.float32)
    nc = _get_program()
    in_maps = [_core_inputs(x, mask, Wq, bq, Wkv, bkv, Wproj, bproj, core)
               for core in range(8)]
    res = bass_utils.run_bass_kernel_spmd(nc, in_maps, core_ids=list(range(8)))
    out = np.empty((B, T, C), np.float32)
    for core in range(8):
        b, h = divmod(core, 2)
        out[b, h * NTOK:(h + 1) * NTOK] = np.asarray(res.results[core]["out"],
                                                     dtype=np.float32)
    return out
